# revision 11
# baseline (speedup 1.0000x reference)
"""Multi-head attention Trainium2 kernel (B=4, S=2048, E=1024, H=16).

Sharding: 8 cores = 4 batch groups x 2-way head tensor-parallel.
Core c handles batch b=c//2 and heads [g*8, g*8+8) with g=c%2.
Each core computes its partial output projection; a 2-way ReduceScatter
pair-sums the partials, so core c ends with tokens [g*1024,(g+1)*1024) of
batch b's final output. The host assembles the full [4,2048,1024] result.

Device layout notes:
- x is transposed on-chip (PE transpose) to xT[e,s] so every matmul
  contracts over the SBUF partition dim.
- Q,K are produced transposed (QT/KT [d,s]) in head pairs stacked to 128
  partitions; scores are computed transposed (scoresT[k,q]) so the exp
  output PT[k,q] feeds the P@V matmul directly as the moving operand.
- V gets a ones-column (65th) so the P@V matmul also emits the softmax
  denominator row for free; normalization happens on the PV output.
- All matmuls run in bf16 (fp32 PSUM accumulate).
"""

import os
import sys

import numpy as np

for _p in ("/opt/trn_rl_repo", "/root/.axon_site/_ro/trn_rl_repo"):
    if os.path.isdir(_p) and _p not in sys.path:
        sys.path.append(_p)

import ml_dtypes  # noqa: E402
from concourse import bacc, mybir, tile  # noqa: E402
from concourse.bass_utils import run_bass_kernel_spmd  # noqa: E402

B, S, E, H, DH = 4, 2048, 1024, 16, 64
N_CORES = 8
TP = 2  # head-parallel factor within a batch
H_LOC = H // TP  # 8 heads per core
EI_LOC = H_LOC * DH  # 512 local rows of the concat dim
S_HALF = S // TP  # 1024 output tokens per core after ReduceScatter
N_SB = S // 128  # 16 token blocks
N_EC = E // 128  # 8 contraction chunks
N_QB = S // 512  # 4 query blocks
N_KB = S // 128  # 16 key blocks
N_HP = H_LOC // 2  # 4 head pairs

BF = mybir.dt.bfloat16
F32 = mybir.dt.float32
EXP = mybir.ActivationFunctionType.Exp
MULT = mybir.AluOpType.mult

_CACHE = {}


def _build():
    nc = bacc.Bacc("TRN2", target_bir_lowering=False, debug=False,
                   num_devices=N_CORES)

    x_in = nc.declare_dram_parameter("x", [S, E], BF, isOutput=False)
    wq_in = nc.declare_dram_parameter("wq", [E, EI_LOC], BF, isOutput=False)
    wk_in = nc.declare_dram_parameter("wk", [E, EI_LOC], BF, isOutput=False)
    wv_in = nc.declare_dram_parameter("wv", [E, EI_LOC], BF, isOutput=False)
    woT_in = nc.declare_dram_parameter("woT", [EI_LOC, E], BF, isOutput=False)
    bob_in = nc.declare_dram_parameter("bob", [128, E], F32, isOutput=False)
    ident_in = nc.declare_dram_parameter("ident", [128, 128], BF, isOutput=False)
    y_out = nc.declare_dram_parameter("y", [S_HALF, E], F32, isOutput=True)

    y_part = nc.dram_tensor("y_part", [S, E], F32)
    y_chunks = [nc.dram_tensor(f"y_chunk{i}", [256, E], F32) for i in range(N_QB)]

    with tile.TileContext(nc) as tc:
        with (
            tc.tile_pool(name="const", bufs=1) as constp,
            tc.tile_pool(name="persist", bufs=1) as persist,
            tc.tile_pool(name="work", bufs=3) as work,
        ):
            ident = constp.tile([128, 128], BF, tag="ident")
            nc.sync.dma_start(ident[:], ident_in[:])
            bob = constp.tile([128, E], F32, tag="bob")
            nc.sync.dma_start(bob[:], bob_in[:])

            wq_t, wk_t, wv_t = [], [], []
            for ec in range(N_EC):
                for lst, src, nm in ((wq_t, wq_in, "wq"), (wk_t, wk_in, "wk"),
                                     (wv_t, wv_in, "wv")):
                    t = constp.tile([128, EI_LOC], BF, tag=f"{nm}{ec}",
                                    name=f"{nm}{ec}")
                    nc.sync.dma_start(t[:], src[ec * 128:(ec + 1) * 128, :])
                    lst.append(t)
            woT_t = []
            for c in range(4):
                t = constp.tile([128, E], BF, tag=f"woT{c}", name=f"woT{c}")
                nc.sync.dma_start(t[:], woT_in[c * 128:(c + 1) * 128, :])
                woT_t.append(t)

            # Two copies of QT/KT with swapped 64-row halves, so the K=64
            # scores matmuls can alternate PE row groups across key blocks
            # (concurrent matmuls + hidden LDWEIGHTS).
            QT = [[persist.tile([128, S], BF, tag=f"QT{p}_{d}",
                                name=f"QT{p}_{d}") for d in range(2)]
                  for p in range(N_HP)]
            KT = [[persist.tile([128, S], BF, tag=f"KT{p}_{d}",
                                name=f"KT{p}_{d}") for d in range(2)]
                  for p in range(N_HP)]
            V = [persist.tile([128, H_LOC, DH + 1], BF, tag=f"V{s}",
                              name=f"V{s}") for s in range(N_SB)]
            CT = [persist.tile([128, S], BF, tag=f"CT{c}", name=f"CT{c}")
                  for c in range(4)]

            # ---- phase A: load x, transpose, projections ----
            with (
                tc.tile_pool(name="xTp", bufs=1) as xTp,
                tc.tile_pool(name="trps", bufs=4, space="PSUM") as trps,
                tc.tile_pool(name="projps", bufs=4, space="PSUM") as projps,
            ):
                xT = [xTp.tile([128, S], BF, tag=f"xT{c}", name=f"xT{c}")
                      for c in range(N_EC)]
                for sb in range(N_SB):
                    xt = work.tile([128, E], BF, tag="xload", name="xload")
                    nc.sync.dma_start(xt[:], x_in[sb * 128:(sb + 1) * 128, :])
                    for ec in range(N_EC):
                        ps = trps.tile([128, 128], BF, tag="trp", name="trp")
                        nc.tensor.transpose(
                            ps[:], xt[:, ec * 128:(ec + 1) * 128], ident[:])
                        nc.vector.tensor_copy(
                            xT[ec][:, sb * 128:(sb + 1) * 128], ps[:])

                # V projection (natural layout [s, h*d]), with ones column
                for sb in range(N_SB):
                    ps = projps.tile([128, EI_LOC], F32, tag="projp", name="vps")
                    for ec in range(N_EC):
                        nc.tensor.matmul(
                            ps[:], xT[ec][:, sb * 128:(sb + 1) * 128],
                            wv_t[ec][:], start=(ec == 0), stop=(ec == N_EC - 1))
                    nc.vector.tensor_copy(V[sb][:, :, 0:DH], ps[:])
                    nc.vector.memset(V[sb][:, :, DH], 1.0)

                # Q/K transposed projections, head pairs stacked on partitions
                for hp in range(N_HP):
                    for qb in range(N_QB):
                        for dst, w in ((QT, wq_t), (KT, wk_t)):
                            ps = projps.tile([128, 512], F32, tag="projp",
                                             name="qkps")
                            for ec in range(N_EC):
                                nc.tensor.matmul(
                                    ps[:],
                                    w[ec][:, hp * 128:(hp + 1) * 128],
                                    xT[ec][:, qb * 512:(qb + 1) * 512],
                                    start=(ec == 0), stop=(ec == N_EC - 1))
                            cols = slice(qb * 512, (qb + 1) * 512)
                            nc.vector.tensor_copy(dst[hp][0][:, cols], ps[:])
                            nc.vector.tensor_copy(
                                dst[hp][1][0:64, cols], ps[64:128, :])
                            nc.vector.tensor_copy(
                                dst[hp][1][64:128, cols], ps[0:64, :])

            # ---- phase B: attention + output projection ----
            # Strips of [128 keys, 1024 queries] (two q-blocks) per exp to
            # amortize ACT per-op overhead; P@V runs one strip behind so PE
            # never waits on the exp of the strip it just produced.
            with (
                tc.tile_pool(name="scps", bufs=2, space="PSUM") as scps,
                tc.tile_pool(name="pvps", bufs=4, space="PSUM") as pvps,
                tc.tile_pool(name="ptp", bufs=3) as ptp,
                tc.tile_pool(name="smallp", bufs=3) as smallp,
                tc.tile_pool(name="youtp", bufs=3) as youtp,
            ):
                inv_sqrt_dh = 1.0 / float(np.sqrt(DH))
                for qp in range(N_QB // 2):  # query pair-blocks of 1024
                    qs2 = slice(qp * 1024, (qp + 1) * 1024)
                    for h in range(H_LOC):
                        hp, hh = h // 2, h % 2
                        pv0 = pvps.tile([DH + 1, 512], F32, tag="pv",
                                        name="pv0")
                        pv1 = pvps.tile([DH + 1, 512], F32, tag="pv",
                                        name="pv1")
                        prev_pt = None
                        for kb in range(N_KB):
                            r = kb % 2  # alternate PE row groups per k block
                            d = 0 if r == hh else 1
                            rows = slice(r * 64, (r + 1) * 64)
                            sp = scps.tile([128, 1024], F32, tag="sc",
                                           name="sc")
                            for half in range(2):
                                nc.tensor.matmul(
                                    sp[:, half * 512:(half + 1) * 512],
                                    KT[hp][d][rows, kb * 128:(kb + 1) * 128],
                                    QT[hp][d][rows,
                                              (2 * qp + half) * 512:
                                              (2 * qp + half + 1) * 512])
                            pt = ptp.tile([128, 1024], BF, tag="pt", name="pt")
                            nc.scalar.activation(pt[:], sp[:], EXP,
                                                 scale=inv_sqrt_dh)
                            if prev_pt is not None:
                                pkb = kb - 1
                                nc.tensor.matmul(
                                    pv0[:], V[pkb][:, h, :],
                                    prev_pt[:, 0:512],
                                    start=(pkb == 0), stop=False)
                                nc.tensor.matmul(
                                    pv1[:], V[pkb][:, h, :],
                                    prev_pt[:, 512:1024],
                                    start=(pkb == 0), stop=False)
                            prev_pt = pt
                        nc.tensor.matmul(pv0[:], V[N_KB - 1][:, h, :],
                                         prev_pt[:, 0:512],
                                         start=False, stop=True)
                        nc.tensor.matmul(pv1[:], V[N_KB - 1][:, h, :],
                                         prev_pt[:, 512:1024],
                                         start=False, stop=True)

                        for half, pv in ((0, pv0), (1, pv1)):
                            qs = slice((2 * qp + half) * 512,
                                       (2 * qp + half + 1) * 512)
                            den = smallp.tile([1, 512], F32, tag="den",
                                              name="den")
                            nc.vector.tensor_copy(den[:], pv[DH:DH + 1, :])
                            denb = smallp.tile([64, 512], F32, tag="denb",
                                               name="denb")
                            nc.gpsimd.partition_broadcast(denb[:], den[:])
                            rec = smallp.tile([64, 512], F32, tag="rec",
                                              name="rec")
                            nc.vector.reciprocal_approx_fast(rec[:], denb[:])
                            ct_rows = slice(hh * 64, (hh + 1) * 64)
                            nc.vector.tensor_tensor(
                                CT[hp][ct_rows, qs], pv[0:DH, :], rec[:], MULT)

                    # output projection + chunked ReduceScatter per q block
                    for qb in (2 * qp, 2 * qp + 1):
                        for sb in range(4 * qb, 4 * qb + 4):
                            ys = scps.tile([128, 1024], F32, tag="sc",
                                           name="ys")
                            for eo in range(2):
                                for c in range(4):
                                    nc.tensor.matmul(
                                        ys[:, eo * 512:(eo + 1) * 512],
                                        CT[c][:, sb * 128:(sb + 1) * 128],
                                        woT_t[c][:, eo * 512:(eo + 1) * 512],
                                        start=(c == 0), stop=(c == 3))
                            yt = youtp.tile([128, E], F32, tag="yt", name="yt")
                            nc.vector.tensor_add(yt[:], ys[:], bob[:])
                            nc.sync.dma_start(
                                y_part[sb * 128:(sb + 1) * 128, :], yt[:])
                        nc.gpsimd.collective_compute(
                            "ReduceScatter", mybir.AluOpType.add,
                            replica_groups=[[0, 1], [2, 3], [4, 5], [6, 7]],
                            ins=[y_part[qb * 512:(qb + 1) * 512, :]],
                            outs=[y_chunks[qb][:]])
                        nc.sync.dma_start(
                            y_out[qb * 256:(qb + 1) * 256, :],
                            y_chunks[qb][:])

    nc.finalize()
    return nc


def _get_nc():
    if "nc" not in _CACHE:
        _CACHE["nc"] = _build()
    return _CACHE["nc"]


def kernel(x, wq, wk, wv, wo, bo):
    nc = _get_nc()
    bf16 = ml_dtypes.bfloat16

    in_maps = []
    ident = np.eye(128, dtype=bf16)
    for c in range(N_CORES):
        b, g = c // TP, c % TP
        h0 = g * H_LOC
        # [H_loc, E, DH] -> [E, H_loc*DH] head-major columns
        wq_l = np.ascontiguousarray(
            wq[h0:h0 + H_LOC].transpose(1, 0, 2).reshape(E, EI_LOC)).astype(bf16)
        wk_l = np.ascontiguousarray(
            wk[h0:h0 + H_LOC].transpose(1, 0, 2).reshape(E, EI_LOC)).astype(bf16)
        wv_l = np.ascontiguousarray(
            wv[h0:h0 + H_LOC].transpose(1, 0, 2).reshape(E, EI_LOC)).astype(bf16)
        # rhs of the output projection: woT[ei, eo] = wo[eo, ei].T slice
        woT_l = np.ascontiguousarray(
            wo[:, g * EI_LOC:(g + 1) * EI_LOC].T).astype(bf16)
        bob = np.broadcast_to(bo.astype(np.float32) / TP, (128, E)).copy()
        in_maps.append({
            "x": x[b].astype(bf16),
            "wq": wq_l, "wk": wk_l, "wv": wv_l, "woT": woT_l,
            "bob": bob, "ident": ident,
        })

    res = run_bass_kernel_spmd(nc, in_maps, list(range(N_CORES)))

    out = np.empty((B, S, E), dtype=np.float32)
    for c in range(N_CORES):
        b, g = c // TP, c % TP
        y = res.results[c]["y"]
        for qb in range(N_QB):
            t0 = qb * 512 + g * 256
            out[b, t0:t0 + 256, :] = y[qb * 256:(qb + 1) * 256, :]
    return out


# revision 13
# speedup vs baseline: 1.1196x; 1.1196x over previous
"""Multi-head attention Trainium2 kernel (B=4, S=2048, E=1024, H=16).

Sharding: 8 cores = 4 batch groups x 2-way head tensor-parallel.
Core c handles batch b=c//2 and heads [g*8, g*8+8) with g=c%2.
Each core computes its partial output projection; a 2-way ReduceScatter
pair-sums the partials, so core c ends with a quarter-slab interleaving of
batch b's final output rows. The host assembles the full [4,2048,1024].

Device layout notes:
- x is transposed on-chip (PE transpose) to xT[e,s] so every matmul
  contracts over the SBUF partition dim.
- Q,K are produced transposed (QT/KT [d,s]) with head pairs stacked on the
  128 partitions (even head rows 0-63, odd head rows 64-127); scores are
  computed transposed (scoresT[k,q]) so the exp output PT[k,q] feeds the
  P@V matmul directly as the moving operand.
- The attention inner loop processes a head PAIR together: the two heads'
  K=64 scores matmuls target disjoint PE row groups, which the PE runs
  concurrently (2.1x measured vs serial same-row-group issue).
- V gets a ones-column (65th) so the P@V matmul also emits the softmax
  denominator row for free; normalization happens on the PV output.
- All matmuls run in bf16 (fp32 PSUM accumulate).
"""

import os
import sys

import numpy as np

for _p in ("/opt/trn_rl_repo", "/root/.axon_site/_ro/trn_rl_repo"):
    if os.path.isdir(_p) and _p not in sys.path:
        sys.path.append(_p)

import ml_dtypes  # noqa: E402
from concourse import bacc, mybir, tile  # noqa: E402
from concourse.bass_utils import run_bass_kernel_spmd  # noqa: E402

B, S, E, H, DH = 4, 2048, 1024, 16, 64
N_CORES = 8
TP = 2  # head-parallel factor within a batch
H_LOC = H // TP  # 8 heads per core
EI_LOC = H_LOC * DH  # 512 local rows of the concat dim
S_HALF = S // TP  # 1024 output tokens per core after ReduceScatter
N_SB = S // 128  # 16 token blocks
N_EC = E // 128  # 8 contraction chunks
N_QB = S // 512  # 4 query blocks
N_KB = S // 128  # 16 key blocks
N_HP = H_LOC // 2  # 4 head pairs

BF = mybir.dt.bfloat16
F32 = mybir.dt.float32
EXP = mybir.ActivationFunctionType.Exp
MULT = mybir.AluOpType.mult

_CACHE = {}


def _build():
    nc = bacc.Bacc("TRN2", target_bir_lowering=False, debug=False,
                   num_devices=N_CORES)

    x_in = nc.declare_dram_parameter("x", [S, E], BF, isOutput=False)
    wq_in = nc.declare_dram_parameter("wq", [E, EI_LOC], BF, isOutput=False)
    wk_in = nc.declare_dram_parameter("wk", [E, EI_LOC], BF, isOutput=False)
    wv_in = nc.declare_dram_parameter("wv", [E, EI_LOC], BF, isOutput=False)
    woT_in = nc.declare_dram_parameter("woT", [EI_LOC, E], BF, isOutput=False)
    bob_in = nc.declare_dram_parameter("bob", [128, E], F32, isOutput=False)
    ident_in = nc.declare_dram_parameter("ident", [128, 128], BF, isOutput=False)
    y_out = nc.declare_dram_parameter("y", [S_HALF, E], F32, isOutput=True)

    y_part = nc.dram_tensor("y_part", [S, E], F32)
    y_chunks = [nc.dram_tensor(f"y_chunk{i}", [256, E], F32)
                for i in range(N_QB)]

    inv_sqrt_dh = 1.0 / float(np.sqrt(DH))

    with tile.TileContext(nc) as tc:
        with (
            tc.tile_pool(name="const", bufs=1) as constp,
            tc.tile_pool(name="persist", bufs=1) as persist,
            tc.tile_pool(name="work", bufs=3) as work,
            # One PSUM pool set shared by all phases (no pool-boundary
            # barrier between projections and attention):
            #   tag "sc": 2 slots x 2 banks  (strips / proj groups / outproj)
            #   tag "pv": 4 slots x 1 bank   (PV accum / PE transposes)
            tc.tile_pool(name="scps", bufs=2, space="PSUM") as scps,
            tc.tile_pool(name="pvps", bufs=4, space="PSUM") as pvps,
            tc.tile_pool(name="ptp", bufs=4) as ptp,
            tc.tile_pool(name="smallp", bufs=4) as smallp,
            tc.tile_pool(name="youtp", bufs=3) as youtp,
        ):
            # ---- constants / weights (x + ident first: transposes are the
            # critical path at startup) ----
            ident = constp.tile([128, 128], BF, tag="ident")
            nc.sync.dma_start(ident[:], ident_in[:])

            xT = [persist.tile([128, S], BF, tag=f"xT{c}", name=f"xT{c}")
                  for c in range(N_EC)]
            QT = [persist.tile([128, S], BF, tag=f"QT{p}", name=f"QT{p}")
                  for p in range(N_HP)]
            KT = [persist.tile([128, S], BF, tag=f"KT{p}", name=f"KT{p}")
                  for p in range(N_HP)]
            V = [persist.tile([128, H_LOC, DH + 1], BF, tag=f"V{s}",
                              name=f"V{s}") for s in range(N_SB)]
            CT = [persist.tile([128, S], BF, tag=f"CT{c}", name=f"CT{c}")
                  for c in range(4)]

            # ---- phase A: load x, transpose, projections ----
            x_tiles = []
            for sb in range(N_SB):
                xt = work.tile([128, E], BF, tag="xload", name="xload")
                nc.sync.dma_start(xt[:], x_in[sb * 128:(sb + 1) * 128, :])
                x_tiles.append((sb, xt))

            wq_t, wk_t, wv_t = [], [], []
            for ec in range(N_EC):
                for lst, src, nm in ((wv_t, wv_in, "wv"), (wq_t, wq_in, "wq"),
                                     (wk_t, wk_in, "wk")):
                    t = constp.tile([128, EI_LOC], BF, tag=f"{nm}{ec}",
                                    name=f"{nm}{ec}")
                    nc.sync.dma_start(t[:], src[ec * 128:(ec + 1) * 128, :])
                    lst.append(t)
            woT_t = []
            for c in range(4):
                t = constp.tile([128, E], BF, tag=f"woT{c}", name=f"woT{c}")
                nc.sync.dma_start(t[:], woT_in[c * 128:(c + 1) * 128, :])
                woT_t.append(t)
            bob = constp.tile([128, E], F32, tag="bob")
            nc.sync.dma_start(bob[:], bob_in[:])

            for sb, xt in x_tiles:
                for ec in range(N_EC):
                    ps = pvps.tile([128, 128], BF, tag="pv", name="trp")
                    nc.tensor.transpose(
                        ps[:], xt[:, ec * 128:(ec + 1) * 128], ident[:])
                    nc.vector.tensor_copy(
                        xT[ec][:, sb * 128:(sb + 1) * 128], ps[:])

            # V projection (natural layout [s, h*d]), with ones column
            for sb in range(N_SB):
                ps = scps.tile([128, EI_LOC], F32, tag="sc", name="vps")
                for ec in range(N_EC):
                    nc.tensor.matmul(
                        ps[:], xT[ec][:, sb * 128:(sb + 1) * 128],
                        wv_t[ec][:], start=(ec == 0), stop=(ec == N_EC - 1))
                nc.vector.tensor_copy(V[sb][:, :, 0:DH], ps[:])
                nc.vector.memset(V[sb][:, :, DH], 1.0)

            # Q/K transposed projections, head pairs stacked on partitions
            for hp in range(N_HP):
                for qb in range(N_QB):
                    for dst, w in ((QT, wq_t), (KT, wk_t)):
                        ps = scps.tile([128, 512], F32, tag="sc",
                                       name="qkps")
                        for ec in range(N_EC):
                            nc.tensor.matmul(
                                ps[:],
                                w[ec][:, hp * 128:(hp + 1) * 128],
                                xT[ec][:, qb * 512:(qb + 1) * 512],
                                start=(ec == 0), stop=(ec == N_EC - 1))
                        nc.vector.tensor_copy(
                            dst[hp][:, qb * 512:(qb + 1) * 512], ps[:])

            # ---- phase B: attention (head pairs) + output projection ----
            for qp in range(N_QB // 2):  # query pair-blocks of 1024
                for hp in range(N_HP):
                    # four PV accumulators: (head in pair) x (q half)
                    pv = [[pvps.tile([DH + 1, 512], F32, tag="pv",
                                     name=f"pv{i}{j}") for j in range(2)]
                          for i in range(2)]
                    prev_pt = None
                    for kb in range(N_KB):
                        ks = slice(kb * 128, (kb + 1) * 128)
                        pts = []
                        for i in range(2):  # head in pair: rows i*64
                            rows = slice(i * 64, (i + 1) * 64)
                            sp = scps.tile([128, 1024], F32, tag="sc",
                                           name="sc")
                            for half in range(2):
                                q5 = slice((2 * qp + half) * 512,
                                           (2 * qp + half + 1) * 512)
                                nc.tensor.matmul(
                                    sp[:, half * 512:(half + 1) * 512],
                                    KT[hp][rows, ks], QT[hp][rows, q5])
                            pt = ptp.tile([128, 1024], BF, tag="pt",
                                          name="pt")
                            nc.scalar.activation(pt[:], sp[:], EXP,
                                                 scale=inv_sqrt_dh)
                            pts.append(pt)
                        if prev_pt is not None:
                            pkb = kb - 1
                            for i in range(2):
                                h = 2 * hp + i
                                for j in range(2):
                                    nc.tensor.matmul(
                                        pv[i][j][:], V[pkb][:, h, :],
                                        prev_pt[i][:, j * 512:(j + 1) * 512],
                                        start=(pkb == 0), stop=False)
                        prev_pt = pts
                    for i in range(2):
                        h = 2 * hp + i
                        for j in range(2):
                            nc.tensor.matmul(
                                pv[i][j][:], V[N_KB - 1][:, h, :],
                                prev_pt[i][:, j * 512:(j + 1) * 512],
                                start=False, stop=True)

                    # drain PV psum to SBUF, then normalize by the ones-row
                    for i in range(2):
                        rows = slice(i * 64, (i + 1) * 64)
                        for j in range(2):
                            qs = slice((2 * qp + j) * 512,
                                       (2 * qp + j + 1) * 512)
                            o65 = smallp.tile([DH + 1, 512], F32, tag="o65",
                                              name="o65")
                            nc.vector.tensor_copy(o65[:], pv[i][j][:])
                            den = smallp.tile([1, 512], F32, tag="den",
                                              name="den")
                            nc.vector.tensor_copy(den[:], o65[DH:DH + 1, :])
                            denb = smallp.tile([64, 512], F32, tag="denb",
                                               name="denb")
                            nc.gpsimd.partition_broadcast(denb[:], den[:])
                            rec = smallp.tile([64, 512], F32, tag="rec",
                                              name="rec")
                            nc.vector.reciprocal_approx_fast(rec[:], denb[:])
                            nc.vector.tensor_tensor(
                                CT[hp][rows, qs], o65[0:DH, :], rec[:], MULT)

                # output projection + chunked ReduceScatter per q block
                for qb in (2 * qp, 2 * qp + 1):
                    for sb in range(4 * qb, 4 * qb + 4):
                        ys = scps.tile([128, 1024], F32, tag="sc", name="ys")
                        for eo in range(2):
                            for c in range(4):
                                nc.tensor.matmul(
                                    ys[:, eo * 512:(eo + 1) * 512],
                                    CT[c][:, sb * 128:(sb + 1) * 128],
                                    woT_t[c][:, eo * 512:(eo + 1) * 512],
                                    start=(c == 0), stop=(c == 3))
                        yt = youtp.tile([128, E], F32, tag="yt", name="yt")
                        nc.vector.tensor_add(yt[:], ys[:], bob[:])
                        nc.sync.dma_start(
                            y_part[sb * 128:(sb + 1) * 128, :], yt[:])
                    nc.gpsimd.collective_compute(
                        "ReduceScatter", mybir.AluOpType.add,
                        replica_groups=[[0, 1], [2, 3], [4, 5], [6, 7]],
                        ins=[y_part[qb * 512:(qb + 1) * 512, :]],
                        outs=[y_chunks[qb][:]])
                    nc.sync.dma_start(
                        y_out[qb * 256:(qb + 1) * 256, :], y_chunks[qb][:])

    nc.finalize()
    return nc


def _get_nc():
    if "nc" not in _CACHE:
        _CACHE["nc"] = _build()
    return _CACHE["nc"]


def _make_in_maps(x, wq, wk, wv, wo, bo):
    bf16 = ml_dtypes.bfloat16
    ident = np.eye(128, dtype=bf16)
    in_maps = []
    for c in range(N_CORES):
        b, g = c // TP, c % TP
        h0 = g * H_LOC
        wq_l = np.ascontiguousarray(
            wq[h0:h0 + H_LOC].transpose(1, 0, 2).reshape(E, EI_LOC)).astype(bf16)
        wk_l = np.ascontiguousarray(
            wk[h0:h0 + H_LOC].transpose(1, 0, 2).reshape(E, EI_LOC)).astype(bf16)
        wv_l = np.ascontiguousarray(
            wv[h0:h0 + H_LOC].transpose(1, 0, 2).reshape(E, EI_LOC)).astype(bf16)
        woT_l = np.ascontiguousarray(
            wo[:, g * EI_LOC:(g + 1) * EI_LOC].T).astype(bf16)
        bob = np.broadcast_to(bo.astype(np.float32) / TP, (128, E)).copy()
        in_maps.append({
            "x": x[b].astype(bf16),
            "wq": wq_l, "wk": wk_l, "wv": wv_l, "woT": woT_l,
            "bob": bob, "ident": ident,
        })
    return in_maps


def _assemble(results):
    out = np.empty((B, S, E), dtype=np.float32)
    for c in range(N_CORES):
        b, g = c // TP, c % TP
        y = results[c]["y"]
        for qb in range(N_QB):
            t0 = qb * 512 + g * 256
            out[b, t0:t0 + 256, :] = y[qb * 256:(qb + 1) * 256, :]
    return out


def kernel(x, wq, wk, wv, wo, bo):
    nc = _get_nc()
    in_maps = _make_in_maps(x, wq, wk, wv, wo, bo)
    res = run_bass_kernel_spmd(nc, in_maps, list(range(N_CORES)))
    return _assemble(res.results)
